# revision 6
# baseline (speedup 1.0000x reference)
"""AdjMatrixGenerator Trainium2 kernel.

Reference computation (B=16, N=256, F=64, H=64):
    a = h @ w1a.T ; c = h @ w1b.T            # [B,N,H] each (w1 split in half)
    z = relu(a[:,i,None,:] + c[:,None,j,:] + b1)   # [B,N,N,H]
    adj = sigmoid(z @ w2.T + b2)             # [B,N,N]
    diagonal forced to 1.

Sharding: data-parallel over batch, 2 batches per core x 8 cores.

The O(B*N*F*H) projections a/c (0.4% of FLOPs) and the final sigmoid are
folded into host-side prep/post; the device kernel does the O(B*N^2*H)
pairwise part, which is elementwise-engine bound. Producer ops are
[128,256] (partitions=(batch,h), free=j) per node i; the per-partition
scalar a_i caps DVE tensor_scalar at 2x mode (196ns i2i) and ACT
activation at 1x (399ns), so FD=256 x 256 ops is the op-shape floor.

  - aT2f [128,256] f32 (= a^T + b1) and cT2 [128,256] bf16 come in via
    DMA split across three rings (sync/vector/scalar), most-urgent first.
  - Nodes processed in PAIRS (2q, 2q+1), one zpair [128,512] bf16 per
    pair. Three producers run concurrently, each pair assigned to one:
    DVE tensor_scalar(add,max) ~196ns/half, ACT activation(Relu, bias)
    ~399ns/half, and GPSIMD tensor_scalar(add,max) as a third engine.
    Assignment is greedy by projected finish time; the last pairs stay
    on DVE (lowest per-pair latency -> shortest tail).
  - Reduce over h with w2: one matmul per pair, column-tiled
    round-robin (pair q -> col-group q%4, PSUM rows 32c+2w+beta,
    lhsT = wbig[:,32-2w:64-2w]) so bunched matmuls run up to 4x
    concurrent in the PE array instead of serializing at 213ns.
  - Logits leave PSUM by DIRECT PSUM->DRAM DMA (f32): no evacuation
    ops on DVE/ACT at all. Group 0's DMA is issued on the idle sync
    ring mid-kernel; group 1 is split into two half DMAs on separate
    rings to shorten the tail. Host applies sigmoid(+b2), the row
    permutation, and diag=1.
"""

import sys

for _p in ("/opt/trn_rl_repo",):
    if _p not in sys.path:
        sys.path.insert(0, _p)

import numpy as np
import ml_dtypes

import concourse.bass as bass
import concourse.tile as tile
from concourse import bacc, mybir
from concourse.bass_utils import run_bass_kernel_spmd

B, N, F, H = 16, 256, 64, 64
NCORES = 8
BLOC = B // NCORES          # batches per core = 2
NG = 2                      # PSUM groups per core (64 pairs each)
PAIRS_PER_G = 64
NPAIRS = NG * PAIRS_PER_G   # 128

F32 = mybir.dt.float32
BF16 = mybir.dt.bfloat16

_COMPILED = None

# per-pair producer costs (ns), used only for the static greedy split
_COST = {"V": 392.0, "A": 798.0, "G": 900.0}
_TAIL_DVE = 6  # last pairs forced onto DVE for a short tail


def _engine_plan():
    """Greedy finish-time assignment of pairs 0..NPAIRS-1 to V/A/G."""
    plan = [None] * NPAIRS
    finish = {"V": 0.0, "A": 0.0, "G": 0.0}
    for q in range(NPAIRS - _TAIL_DVE):
        e = min(finish, key=lambda k: finish[k] + _COST[k])
        plan[q] = e
        finish[e] += _COST[e]
    for q in range(NPAIRS - _TAIL_DVE, NPAIRS):
        plan[q] = "V"
    return plan


def _build(use_gpsimd=True):
    nc = bacc.Bacc("TRN2", target_bir_lowering=False, debug=False,
                   enable_asserts=False, num_devices=NCORES)

    aT2f_d = nc.dram_tensor("aT2f", [128, N], F32, kind="ExternalInput").ap()
    cT2_d = nc.dram_tensor("cT2", [128, N], BF16, kind="ExternalInput").ap()
    wbig_d = nc.dram_tensor("wbig", [128, 64], BF16, kind="ExternalInput").ap()
    out_d = nc.dram_tensor("out", [NG, 128, 512], BF16, kind="ExternalOutput").ap()

    Relu = mybir.ActivationFunctionType.Relu
    Identity = mybir.ActivationFunctionType.Identity
    ADD = mybir.AluOpType.add
    MAX = mybir.AluOpType.max

    plan = _engine_plan()
    if not use_gpsimd:
        plan = ["A" if e == "G" else e for e in plan]

    with tile.TileContext(nc) as tc:
        with (
            tc.tile_pool(name="const", bufs=1) as cpool,
            tc.tile_pool(name="z", bufs=24) as zpool,
            tc.tile_pool(name="sig", bufs=2) as spool,
            tc.tile_pool(name="pmain", bufs=2, space=bass.MemorySpace.PSUM) as ppm,
        ):
            # ---- inputs across three DMA rings, most-urgent first. Every
            # producer op streams all of cT2, so its halves fly in parallel
            # on the two HWDGE rings; the first aT2f columns ride the
            # gpsimd SWDGE ring (Q7 is idle until cT2 lands anyway).
            aT2f = cpool.tile([128, N], F32)   # a^T + b1 (f32 scalar/bias)
            cT2 = cpool.tile([128, N], BF16)   # c^T bf16, streamed by all 3
            wbig = cpool.tile([128, 64], BF16)
            nc.sync.dma_start(cT2[:, 0:128], cT2_d[:, 0:128])
            nc.scalar.dma_start(cT2[:, 128:256], cT2_d[:, 128:256])
            nc.gpsimd.dma_start(aT2f[:, 0:32], aT2f_d[:, 0:32])
            nc.sync.dma_start(aT2f[:, 32:144], aT2f_d[:, 32:144])
            nc.scalar.dma_start(wbig[:], wbig_d)
            nc.scalar.dma_start(aT2f[:, 144:256], aT2f_d[:, 144:256])

            pending = None   # previous group's PSUM awaiting evacuation
            last_act = None  # most recent ACT relu (ordering anchor)
            for g in range(NG):
                psum_t = ppm.tile([128, 512], F32)
                for q in range(PAIRS_PER_G):
                    qg = g * PAIRS_PER_G + q
                    zpair = zpool.tile([128, 512], BF16)
                    eng = plan[qg]
                    for half in range(2):
                        i = 2 * qg + half
                        dst = zpair[:, 256 * half:256 * half + 256]
                        if eng == "A":
                            last_act = nc.scalar.activation(
                                dst, cT2[:], Relu,
                                bias=aT2f[:, i:i + 1], scale=1.0)
                        elif eng == "G":
                            nc.gpsimd.tensor_scalar(dst, cT2[:],
                                                    aT2f[:, i:i + 1], 0.0,
                                                    op0=ADD, op1=MAX)
                        else:
                            nc.vector.tensor_scalar(dst, cT2[:],
                                                    aT2f[:, i:i + 1], 0.0,
                                                    op0=ADD, op1=MAX)
                    # column-tiled reduce: consecutive pairs round-robin the
                    # 4 col-groups so bunched matmuls run concurrently.
                    # Pair q -> col c=q%4, slot w=q//4, PSUM rows 32c+2w+beta.
                    c = q % 4
                    w = q // 4
                    nc.tensor.matmul(
                        psum_t[32 * c:32 * c + 32, :],
                        wbig[:, 32 - 2 * w:64 - 2 * w],
                        zpair[:],
                        start=(q < 4), stop=(q >= PAIRS_PER_G - 4),
                        tile_position=(0, 32 * c))
                    if q == 10 and pending is not None:
                        # Deferred PSUM->SBUF copy of the PREVIOUS group's
                        # logits (Identity: same table set as Relu), kept
                        # behind ~10 pairs of this group's relus via an
                        # ordering edge so it doesn't stall the boundary.
                        dsig = spool.tile([128, 512], BF16)
                        si = nc.scalar.activation(dsig[:], pending[:],
                                                  Identity, scale=1.0)
                        tile.add_dep_helper(
                            getattr(si, 'ins', si),
                            getattr(last_act, 'ins', last_act),
                            sync=False,
                            reason="defer prev-group evacuation past relus")
                        nc.sync.dma_start(out_d[g - 1], dsig[:])
                        pending = None
                if g < NG - 1:
                    pending = psum_t
                    continue
                sig = spool.tile([128, 512], BF16)
                # last group: evacuate both halves concurrently, one on DVE
                # (tensor_copy) and one on ACT (Identity), each DMAed on its
                # own ring, to minimize the tail after the final reduce.
                nc.vector.tensor_copy(sig[:, 0:256], psum_t[:, 0:256])
                nc.sync.dma_start(out_d[g][:, 0:256], sig[:, 0:256])
                nc.scalar.activation(sig[:, 256:512], psum_t[:, 256:512],
                                     Identity, scale=1.0)
                nc.scalar.dma_start(out_d[g][:, 256:512], sig[:, 256:512])

    nc.compile()
    return nc


def _get_compiled():
    global _COMPILED
    if _COMPILED is None:
        _COMPILED = _build()
    return _COMPILED


def _prep_in_maps(hidden_state, w1, b1, w2, b2):
    hidden_state = np.asarray(hidden_state, dtype=np.float32)
    w1 = np.asarray(w1, dtype=np.float32)
    b1 = np.asarray(b1, dtype=np.float32)
    w2 = np.asarray(w2, dtype=np.float32)

    w1a, w1b = w1[:, :F], w1[:, F:]                   # [H, F] each
    # a^T + b1 / c^T with partitions = (batch, h): row 64*beta + h, col = node
    a = hidden_state @ w1a.T + b1                     # [B, N, H]
    c = hidden_state @ w1b.T                          # [B, N, H]
    # 64-col band of the shifted-window weight matrix: nonzero w2 columns
    # sit at band index 32 (batch 0) / 33 (batch 1); lhsT slice for slot w
    # is wbig[:, 32-2w : 64-2w].
    wbig = np.zeros((128, 64), dtype=ml_dtypes.bfloat16)
    wbig[0:64, 32] = w2[0].astype(ml_dtypes.bfloat16)
    wbig[64:128, 33] = w2[0].astype(ml_dtypes.bfloat16)

    in_maps = []
    for k in range(NCORES):
        sa = a[BLOC * k:BLOC * (k + 1)]               # [2, 256, 64]
        sc = c[BLOC * k:BLOC * (k + 1)]
        aT2f = np.ascontiguousarray(
            sa.transpose(0, 2, 1).reshape(2 * H, N)).astype(np.float32)
        cT2 = np.ascontiguousarray(
            sc.transpose(0, 2, 1).reshape(2 * H, N)).astype(ml_dtypes.bfloat16)
        in_maps.append({"aT2f": aT2f, "cT2": cT2, "wbig": wbig})
    return in_maps


def kernel(hidden_state, w1, b1, w2, b2):
    nc = _get_compiled()
    in_maps = _prep_in_maps(hidden_state, w1, b1, w2, b2)
    res = run_bass_kernel_spmd(nc, in_maps, core_ids=list(range(NCORES)))
    b2 = np.asarray(b2, dtype=np.float32)
    out = np.empty((B, N, N), dtype=np.float32)
    for k in range(NCORES):
        flat = np.asarray(res.results[k]["out"]).astype(np.float32)
        # psum row p = 32c + 2w + beta for pair q = 4w + c
        # -> i = 128 g + 2 q + half = 128 g + 8 w + 2 c + half
        arr = flat.reshape(NG, 4, 16, 2, 2, N)        # g, c, w, beta, half, j
        arr = arr.transpose(3, 0, 2, 1, 4, 5).reshape(BLOC, N, N)
        out[BLOC * k:BLOC * (k + 1)] = arr
    # sigmoid(+b2) on host (f32, better precision than the ACT spline)
    out = 1.0 / (1.0 + np.exp(-(out + b2[0])))
    idx = np.arange(N)
    out[:, idx, idx] = 1.0
    return out


# revision 7
# speedup vs baseline: 3.6152x; 3.6152x over previous
"""AdjMatrixGenerator Trainium2 kernel.

Reference computation (B=16, N=256, F=64, H=64):
    a = h @ w1a.T ; c = h @ w1b.T            # [B,N,H] each (w1 split in half)
    z = relu(a[:,i,None,:] + c[:,None,j,:] + b1)   # [B,N,N,H]
    adj = sigmoid(z @ w2.T + b2)             # [B,N,N]
    diagonal forced to 1.

Sharding: data-parallel over batch, 2 batches per core x 8 cores.

The O(B*N*F*H) projections a/c (0.4% of FLOPs) and the final sigmoid are
folded into host-side prep/post; the device kernel does the O(B*N^2*H)
pairwise part, which is elementwise-engine bound. Producer ops are
[128,256] (partitions=(batch,h), free=j) per node i; the per-partition
scalar a_i caps DVE tensor_scalar at 2x mode (196ns i2i) and ACT
activation at 1x, so FD=256 x 256 ops is the op-shape floor. GPSIMD's
tensor_scalar ucode measures ~3.9us/op on HW and stalls DVE via the
shared SBUF port -- unusable as a third producer.

  - aT2f [128,256] f32 (= a^T + b1) and cT2 [128,256] bf16 come in via
    DMA over the two HWDGE rings (sync/scalar) + the gpsimd SWDGE ring
    for late aT2f columns; most-urgent bytes first.
  - cT2 is also copied once into PSUM (one identity matmul) so ACT's
    relu ops stream from PSUM: ScalarE PSUM-source overhead is much
    lower than SBUF-source (172 vs 224 cycles in the errata table).
  - Nodes processed in PAIRS (2q, 2q+1), one zpair [128,512] bf16 per
    pair. Producer split per PAIR between DVE tensor_scalar(add,max)
    and ACT activation(Relu, bias); assignment greedy by projected
    finish time, last pairs on DVE (lowest latency -> short tail).
  - Reduce over h with w2: one matmul per pair, column-tiled
    round-robin (pair q -> col-group q%4, PSUM rows 32c+2w+beta,
    lhsT = wbig[:,32-2w:64-2w]) so bunched matmuls run up to 4x
    concurrent in the PE array instead of serializing at 213ns.
  - Logits leave PSUM via an Identity activation (group 0, deferred
    ~10 pairs into group 1 behind an ordering edge); the last group is
    evacuated as two parallel chunks (DVE tensor_copy + ACT Identity,
    split by their measured PSUM-evac rates), each DMAed on its own
    ring. Host applies sigmoid(+b2), row permutation, and diag=1.
"""

import sys

for _p in ("/opt/trn_rl_repo",):
    if _p not in sys.path:
        sys.path.insert(0, _p)

import numpy as np
import ml_dtypes

import concourse.bass as bass
import concourse.tile as tile
from concourse import bacc, mybir
from concourse.bass_utils import run_bass_kernel_spmd

B, N, F, H = 16, 256, 64, 64
NCORES = 8
BLOC = B // NCORES          # batches per core = 2
NG = 2                      # PSUM groups per core (64 pairs each)
PAIRS_PER_G = 64
NPAIRS = NG * PAIRS_PER_G   # 128

F32 = mybir.dt.float32
BF16 = mybir.dt.bfloat16

_COMPILED = None

# per-pair producer costs (ns) for the static greedy split; ACT cost
# assumes the PSUM-source path. Retuned from traces.
_COST = {"V": 392.0, "A": 640.0}
_ACT_EXTRA = 950.0   # group-0 evac + final-chunk Identity ride on ACT
_TAIL_DVE = 6        # last pairs forced onto DVE for a short tail


def _engine_plan():
    """Greedy finish-time assignment of pairs 0..NPAIRS-1 to V/A."""
    plan = [None] * NPAIRS
    finish = {"V": 0.0, "A": _ACT_EXTRA}
    for q in range(NPAIRS - _TAIL_DVE):
        e = min(finish, key=lambda k: finish[k] + _COST[k])
        plan[q] = e
        finish[e] += _COST[e]
    for q in range(NPAIRS - _TAIL_DVE, NPAIRS):
        plan[q] = "V"
    return plan


def _build(act_from_psum=True):
    nc = bacc.Bacc("TRN2", target_bir_lowering=False, debug=False,
                   enable_asserts=False, num_devices=NCORES)

    aT2f_d = nc.dram_tensor("aT2f", [128, N], F32, kind="ExternalInput").ap()
    cT2_d = nc.dram_tensor("cT2", [128, N], BF16, kind="ExternalInput").ap()
    wbig_d = nc.dram_tensor("wbig", [128, 64], BF16, kind="ExternalInput").ap()
    ident_d = nc.dram_tensor("ident", [128, 128], BF16, kind="ExternalInput").ap()
    out_d = nc.dram_tensor("out", [NG, 128, 512], BF16, kind="ExternalOutput").ap()

    Relu = mybir.ActivationFunctionType.Relu
    Identity = mybir.ActivationFunctionType.Identity
    ADD = mybir.AluOpType.add
    MAX = mybir.AluOpType.max

    plan = _engine_plan()

    with tile.TileContext(nc) as tc:
        with (
            tc.tile_pool(name="const", bufs=1) as cpool,
            tc.tile_pool(name="z", bufs=24) as zpool,
            tc.tile_pool(name="sig", bufs=2) as spool,
            tc.tile_pool(name="pmain", bufs=2, space=bass.MemorySpace.PSUM) as ppm,
            tc.tile_pool(name="pc", bufs=1, space=bass.MemorySpace.PSUM) as ppc,
        ):
            # ---- inputs, most-urgent first. Every producer op streams all
            # of cT2, so its halves fly in parallel on the two HWDGE rings;
            # a small leading aT2f chunk on sync unblocks the first DVE
            # scalars; late aT2f columns ride the gpsimd SWDGE ring.
            aT2f = cpool.tile([128, N], F32)   # a^T + b1 (f32 scalar/bias)
            cT2 = cpool.tile([128, N], BF16)   # c^T bf16
            wbig = cpool.tile([128, 64], BF16)
            ident = cpool.tile([128, 128], BF16)
            nc.sync.dma_start(aT2f[:, 0:16], aT2f_d[:, 0:16])
            nc.sync.dma_start(cT2[:, 0:128], cT2_d[:, 0:128])
            nc.scalar.dma_start(cT2[:, 128:256], cT2_d[:, 128:256])
            nc.scalar.dma_start(ident[:], ident_d)
            nc.sync.dma_start(aT2f[:, 16:64], aT2f_d[:, 16:64])
            nc.scalar.dma_start(wbig[:], wbig_d)
            nc.gpsimd.dma_start(aT2f[:, 64:160], aT2f_d[:, 64:160])
            nc.scalar.dma_start(aT2f[:, 160:256], aT2f_d[:, 160:256])

            # one-time copy of cT2 into PSUM: ACT's relu ops then stream
            # from the faster PSUM source while DVE streams the SBUF copy.
            cT2P = ppc.tile([128, N], F32)
            if act_from_psum:
                nc.tensor.matmul(cT2P[:], ident[:], cT2[:],
                                 start=True, stop=True)
            act_src = cT2P if act_from_psum else cT2

            pending = None   # previous group's PSUM awaiting evacuation
            last_act = None  # most recent ACT relu (ordering anchor)
            for g in range(NG):
                psum_t = ppm.tile([128, 512], F32)
                for q in range(PAIRS_PER_G):
                    qg = g * PAIRS_PER_G + q
                    zpair = zpool.tile([128, 512], BF16)
                    for half in range(2):
                        i = 2 * qg + half
                        dst = zpair[:, 256 * half:256 * half + 256]
                        if plan[qg] == "A":
                            last_act = nc.scalar.activation(
                                dst, act_src[:], Relu,
                                bias=aT2f[:, i:i + 1], scale=1.0)
                        else:
                            nc.vector.tensor_scalar(dst, cT2[:],
                                                    aT2f[:, i:i + 1], 0.0,
                                                    op0=ADD, op1=MAX)
                    # column-tiled reduce: consecutive pairs round-robin the
                    # 4 col-groups so bunched matmuls run concurrently.
                    # Pair q -> col c=q%4, slot w=q//4, PSUM rows 32c+2w+beta.
                    c = q % 4
                    w = q // 4
                    nc.tensor.matmul(
                        psum_t[32 * c:32 * c + 32, :],
                        wbig[:, 32 - 2 * w:64 - 2 * w],
                        zpair[:],
                        start=(q < 4), stop=(q >= PAIRS_PER_G - 4),
                        tile_position=(0, 32 * c))
                    if q == 10 and pending is not None:
                        # Deferred PSUM->SBUF copy of the PREVIOUS group's
                        # logits (Identity: same table set as Relu), kept
                        # behind ~10 pairs of this group's relus via an
                        # ordering edge so it doesn't stall the boundary.
                        dsig = spool.tile([128, 512], BF16)
                        si = nc.scalar.activation(dsig[:], pending[:],
                                                  Identity, scale=1.0)
                        tile.add_dep_helper(
                            getattr(si, 'ins', si),
                            getattr(last_act, 'ins', last_act),
                            sync=False,
                            reason="defer prev-group evacuation past relus")
                        nc.sync.dma_start(out_d[g - 1], dsig[:])
                        pending = None
                if g < NG - 1:
                    pending = psum_t
                    continue
                sig = spool.tile([128, 512], BF16)
                # last group: evacuate in two parallel chunks sized by the
                # engines' measured PSUM-evac rates (DVE ~1.66ns/col,
                # ACT ~0.93ns/col), each DMAed on its own ring.
                nc.vector.tensor_copy(sig[:, 0:192], psum_t[:, 0:192])
                nc.sync.dma_start(out_d[g][:, 0:192], sig[:, 0:192])
                nc.scalar.activation(sig[:, 192:512], psum_t[:, 192:512],
                                     Identity, scale=1.0)
                nc.scalar.dma_start(out_d[g][:, 192:512], sig[:, 192:512])

    nc.compile()
    return nc


def _get_compiled():
    global _COMPILED
    if _COMPILED is None:
        _COMPILED = _build()
    return _COMPILED


def _prep_in_maps(hidden_state, w1, b1, w2, b2):
    hidden_state = np.asarray(hidden_state, dtype=np.float32)
    w1 = np.asarray(w1, dtype=np.float32)
    b1 = np.asarray(b1, dtype=np.float32)
    w2 = np.asarray(w2, dtype=np.float32)

    w1a, w1b = w1[:, :F], w1[:, F:]                   # [H, F] each
    # a^T + b1 / c^T with partitions = (batch, h): row 64*beta + h, col = node
    a = hidden_state @ w1a.T + b1                     # [B, N, H]
    c = hidden_state @ w1b.T                          # [B, N, H]
    # 64-col band of the shifted-window weight matrix: nonzero w2 columns
    # sit at band index 32 (batch 0) / 33 (batch 1); lhsT slice for slot w
    # is wbig[:, 32-2w : 64-2w].
    wbig = np.zeros((128, 64), dtype=ml_dtypes.bfloat16)
    wbig[0:64, 32] = w2[0].astype(ml_dtypes.bfloat16)
    wbig[64:128, 33] = w2[0].astype(ml_dtypes.bfloat16)
    ident = np.eye(128, dtype=ml_dtypes.bfloat16)

    in_maps = []
    for k in range(NCORES):
        sa = a[BLOC * k:BLOC * (k + 1)]               # [2, 256, 64]
        sc = c[BLOC * k:BLOC * (k + 1)]
        aT2f = np.ascontiguousarray(
            sa.transpose(0, 2, 1).reshape(2 * H, N)).astype(np.float32)
        cT2 = np.ascontiguousarray(
            sc.transpose(0, 2, 1).reshape(2 * H, N)).astype(ml_dtypes.bfloat16)
        in_maps.append({"aT2f": aT2f, "cT2": cT2, "wbig": wbig,
                        "ident": ident})
    return in_maps


def kernel(hidden_state, w1, b1, w2, b2):
    nc = _get_compiled()
    in_maps = _prep_in_maps(hidden_state, w1, b1, w2, b2)
    res = run_bass_kernel_spmd(nc, in_maps, core_ids=list(range(NCORES)))
    b2 = np.asarray(b2, dtype=np.float32)
    out = np.empty((B, N, N), dtype=np.float32)
    for k in range(NCORES):
        # bf16 logits (values ~1e-2; bf16 rounding adds ~1e-5 rel on adj)
        flat = np.asarray(res.results[k]["out"]).astype(np.float32)
        # psum row p = 32c + 2w + beta for pair q = 4w + c
        # -> i = 128 g + 2 q + half = 128 g + 8 w + 2 c + half
        arr = flat.reshape(NG, 4, 16, 2, 2, N)        # g, c, w, beta, half, j
        arr = arr.transpose(3, 0, 2, 1, 4, 5).reshape(BLOC, N, N)
        out[BLOC * k:BLOC * (k + 1)] = arr
    # sigmoid(+b2) on host (f32, better precision than the ACT spline)
    out = 1.0 / (1.0 + np.exp(-(out + b2[0])))
    idx = np.arange(N)
    out[:, idx, idx] = 1.0
    return out
